# revision 9
# baseline (speedup 1.0000x reference)
"""Trainium2 Bass kernel for nn_CustomLLamaModel (RMSNorm + QK proj + RoPE + causal QK^T).

Sharding: 8 cores, tensor-parallel over attention heads. Core i computes q heads
4i..4i+3 and kv head i (GQA groups align exactly with the 8 cores, so no
collectives are needed). Each core receives the full (bf16-cast) activations and
its weight shard, and writes its 4 heads' [2048, 2048] score matrices.

Device pipeline per core (all matmuls bf16, PSUM f32):
  - x row-tiles [128, 4096]: bn_stats/bn_aggr -> mean(x^2) -> r = rsqrt(mean+eps)
  - transpose x via PE matmuls (lhsT=x chunk, rhs=I) -> xT [4096, 512-chunk]
  - r is folded into the RoPE cos/sin tables (rope is linear, rope(r*v)=r*rope(v)),
    so projections run on the UN-normalized xT and normalization comes out in rope
  - projections: qT/kT = W^T @ xT accumulated over 32 K-chunks
  - rope: rotate-half via two SBUF->SBUF partition-shift DMAs; sign folded in table
  - scores: only lower-triangle 512-blocks are computed; the diagonal block gets a
    precomputed triangular min_f mask added at PSUM eviction; the upper-triangle
    region is written from a constant min_f SBUF tile (exact: score+min_f == min_f
    in f32); 1/sqrt(HD) and the RMSNorm gain g are folded into Wq/Wk on the host.
"""

import os
import sys

sys.path.insert(0, "/opt/trn_rl_repo")

import math
import numpy as np
import ml_dtypes

_THIS_DIR = os.path.dirname(os.path.abspath(__file__))
if _THIS_DIR not in sys.path:
    sys.path.insert(0, _THIS_DIR)

try:
    import axon_profile_shim

    axon_profile_shim.install()
except Exception:
    pass

import concourse.bass as bass
import concourse.mybir as mybir
import concourse.tile as tile
from concourse import bacc
from concourse.bass_utils import run_bass_kernel_spmd

B, S, D = 1, 2048, 4096
H, KVH, HD = 32, 8, 128
ROPE_THETA = 10000.0
RMS_EPS = 1e-5
NCORES = 8
HPC = H // NCORES  # q heads per core = 4
P = 128
NRT = S // P  # 16 row tiles
SC = 512  # seq chunk
NSC = S // SC  # 4 chunks
KO = D // P  # 32 contraction chunks
MIN_F = float(np.finfo(np.float32).min)

BF16 = mybir.dt.bfloat16
F32 = mybir.dt.float32

_cache = {}


def _build_nc():
    """Build + compile the per-core NEFF (same program for all 8 cores)."""
    nc = bacc.Bacc(
        "TRN2",
        target_bir_lowering=False,
        debug=False,
        enable_asserts=True,
        num_devices=NCORES,
    )
    xb = nc.dram_tensor("xb", [S, D], BF16, kind="ExternalInput")
    wq = nc.dram_tensor("wq", [D, HPC * HD], BF16, kind="ExternalInput")
    wk = nc.dram_tensor("wk", [D, HD], BF16, kind="ExternalInput")
    cos_d = nc.dram_tensor("cos", [P, S], BF16, kind="ExternalInput")
    sinn_d = nc.dram_tensor("sinn", [P, S], BF16, kind="ExternalInput")
    tri_d = nc.dram_tensor("tri", [P, SC], F32, kind="ExternalInput")
    identb_d = nc.dram_tensor("identb", [P, P], BF16, kind="ExternalInput")
    identf_d = nc.dram_tensor("identf", [P, P], F32, kind="ExternalInput")
    out = nc.dram_tensor("out", [HPC, S, S], F32, kind="ExternalOutput")

    with tile.TileContext(nc) as tc:
        _emit(nc, tc, xb, wq, wk, cos_d, sinn_d, tri_d, identb_d, identf_d, out)
    nc.compile()
    return nc


def _emit(nc, tc, xb, wq, wk, cos_d, sinn_d, tri_d, identb_d, identf_d, out):
    from contextlib import ExitStack

    ctx = ExitStack()
    with ctx:
        singles = ctx.enter_context(tc.tile_pool(name="singles", bufs=1))
        xrow_p = ctx.enter_context(tc.tile_pool(name="xrow", bufs=2))
        xt_p = ctx.enter_context(tc.tile_pool(name="xt", bufs=2))
        stat_p = ctx.enter_context(tc.tile_pool(name="stat", bufs=4))
        qt_p = ctx.enter_context(tc.tile_pool(name="qt", bufs=2))
        rot_p = ctx.enter_context(tc.tile_pool(name="rot", bufs=2))
        rbc_p = ctx.enter_context(tc.tile_pool(name="rbc", bufs=2))
        ev_p = ctx.enter_context(tc.tile_pool(name="ev", bufs=3))
        ps_tr = ctx.enter_context(tc.tile_pool(name="ps_tr", bufs=2, space="PSUM"))
        ps_pr = ctx.enter_context(tc.tile_pool(name="ps_pr", bufs=2, space="PSUM"))
        ps_sc = ctx.enter_context(tc.tile_pool(name="ps_sc", bufs=3, space="PSUM"))

        # ---- resident constants / persistent tensors ----
        wq_sb = singles.tile([P, KO, HPC * HD], BF16)
        nc.sync.dma_start(wq_sb[:], wq.rearrange("(ko p) m -> p ko m", p=P))
        wk_sb = singles.tile([P, KO, HD], BF16)
        nc.sync.dma_start(wk_sb[:], wk.rearrange("(ko p) m -> p ko m", p=P))
        cos_sb = singles.tile([P, S], BF16)
        nc.sync.dma_start(cos_sb[:], cos_d[:])
        sinn_sb = singles.tile([P, S], BF16)
        nc.sync.dma_start(sinn_sb[:], sinn_d[:])
        tri_sb = singles.tile([P, SC], F32)
        nc.sync.dma_start(tri_sb[:], tri_d[:])
        identb = singles.tile([P, P], BF16)
        nc.sync.dma_start(identb[:], identb_d[:])
        identf = singles.tile([P, P], F32)
        nc.sync.dma_start(identf[:], identf_d[:])
        minf_sb = singles.tile([P, S - P], F32)
        nc.vector.memset(minf_sb[:], MIN_F)
        eps_sb = singles.tile([P, 1], F32)
        nc.vector.memset(eps_sb[:], RMS_EPS)

        r_all = singles.tile([P, NRT], F32)  # r_all[p, t] = rsqrt for seq 128t+p
        cos_r = singles.tile([P, S], BF16)  # r-folded rope tables
        sin_r = singles.tile([P, S], BF16)
        q_ro = singles.tile([P, HPC, S], BF16)  # roped qT per head
        k_ro = singles.tile([P, S], BF16)  # roped kT
        r_row = singles.tile([1, SC], F32)
        r_psT = singles.tile([32, P], F32)

        # Constant min_f (upper-triangle) writes have no compute dependency:
        # issue them on the GPSIMD/SWDGE ring (separate FIFO from the SP ring
        # carrying input loads and the ACT ring carrying computed outputs) so
        # they fill DMA idle time instead of sitting on the critical-path tail.
        # Coalesced across the 4 heads per row-tile: dest reordered (r, h, c),
        # source re-reads the minf tile 4x via a 0-stride dim.
        def emit_const_dma(i):
            W = (i + 1) * P
            if W >= S:
                return
            for h in range(HPC):
                nc.gpsimd.dma_start(out[h, i * P : (i + 1) * P, W:S],
                                    minf_sb[:, : S - W])

        for i in range(4):
            emit_const_dma(i)

        ev_dve = True  # alternate engines for copies

        for c in range(NSC):
            sl = slice(c * SC, (c + 1) * SC)
            xt_c = xt_p.tile([P, KO, SC], BF16, tag="xt")
            # ---- load 4 row-tiles, stats, transpose ----
            for tt in range(4):
                t = 4 * c + tt
                xrow = xrow_p.tile([P, D], BF16, tag="xrow")
                nc.sync.dma_start(xrow[:], xb[t * P : (t + 1) * P, :])
                # stats: mean(x^2) = var + mean^2 over the 4096 free dim
                stats = stat_p.tile([P, 8, nc.vector.BN_STATS_DIM], F32, tag="bnst")
                xv = xrow[:].rearrange("p (n f) -> p n f", f=512)
                for sg in range(8):
                    nc.vector.bn_stats(out=stats[:, sg, :], in_=xv[:, sg, :])
                mv = stat_p.tile([P, nc.vector.BN_AGGR_DIM], F32, tag="bnmv")
                nc.vector.bn_aggr(out=mv[:], in_=stats[:])
                msq = stat_p.tile([P, 1], F32, tag="msq")
                nc.vector.tensor_mul(msq[:], mv[:, 0:1], mv[:, 0:1])
                nc.vector.tensor_add(msq[:], msq[:], mv[:, 1:2])
                std = stat_p.tile([P, 1], F32, tag="std")
                nc.scalar.activation(
                    out=std[:], in_=msq[:],
                    func=mybir.ActivationFunctionType.Sqrt,
                    bias=eps_sb[:], scale=1.0,
                )
                nc.vector.reciprocal(out=r_all[:, t : t + 1], in_=std[:])
                # transpose this row-tile: 32 chunk transposes in groups of 4
                for dg in range(8):
                    ps = ps_tr.tile([P, 4 * P], F32, tag="pstr")
                    for u in range(4):
                        d = 4 * dg + u
                        nc.tensor.matmul(
                            ps[:, u * P : (u + 1) * P],
                            xrow[:, d * P : (d + 1) * P],
                            identb[:],
                            start=True, stop=True,
                        )
                    dst = xt_c[:, 4 * dg : 4 * dg + 4, tt * P : (tt + 1) * P]
                    src = ps[:].rearrange("p (a b) -> p a b", a=4)
                    if ev_dve:
                        nc.vector.tensor_copy(dst, src)
                    else:
                        nc.scalar.copy(dst, src)
                    ev_dve = not ev_dve

            # ---- r broadcast chain for this chunk ----
            ps4full = ps_pr.tile([P, SC], F32, tag="pspr")
            ps4 = ps4full[0:4, 0:P]
            nc.tensor.matmul(ps4, r_all[:, 4 * c : 4 * c + 4], identf[:],
                             start=True, stop=True)
            nc.vector.tensor_copy(r_psT[0:4, :], ps4)
            for tt in range(4):
                nc.sync.dma_start(r_row[0:1, tt * P : (tt + 1) * P],
                                  r_psT[tt : tt + 1, :])
            r_bc = rbc_p.tile([P, SC], F32, tag="rbc")
            nc.gpsimd.partition_broadcast(r_bc[:], r_row[0:1, :])
            nc.vector.tensor_mul(cos_r[:, sl], cos_sb[:, sl], r_bc[:])
            nc.vector.tensor_mul(sin_r[:, sl], sinn_sb[:, sl], r_bc[:])
            # trickle out the remaining constant-region writes (Q7 emission is
            # serial with partition_broadcast, so don't front-load all of them)
            for i in range(4 + 3 * c, min(4 + 3 * c + 3, NRT)):
                emit_const_dma(i)

            # ---- projections + rope ----
            # out-tiles: (wq_sb, m=0..3 -> q_ro[:, m, :]), (wk_sb -> k_ro)
            proj_list = [(wq_sb, m, q_ro[:, m, :]) for m in range(HPC)]
            proj_list.append((wk_sb, 0, k_ro[:]))
            for w_sb, m, dest in proj_list:
                ps = ps_pr.tile([P, SC], F32, tag="pspr")
                for ko in range(KO):
                    nc.tensor.matmul(
                        ps[:],
                        w_sb[:, ko, m * P : (m + 1) * P],
                        xt_c[:, ko, :],
                        start=(ko == 0), stop=(ko == KO - 1),
                    )
                qt = qt_p.tile([P, SC], BF16, tag="qt")
                nc.scalar.copy(qt[:], ps[:])  # evict f32->bf16 on ACT
                rot = rot_p.tile([P, SC], BF16, tag="rot")
                nc.sync.dma_start(rot[0:64, :], qt[64:128, :])
                nc.sync.dma_start(rot[64:128, :], qt[0:64, :])
                nc.vector.tensor_mul(rot[:], rot[:], sin_r[:, sl])
                nc.vector.tensor_mul(dest[:, sl], qt[:], cos_r[:, sl])
                nc.vector.tensor_add(dest[:, sl], dest[:, sl], rot[:])

            # ---- scores for q row-tiles of this chunk ----
            for h in range(HPC):
                for tt in range(4):
                    i = 4 * c + tt
                    W = (i + 1) * P
                    nch = (W + SC - 1) // SC
                    ev = ev_p.tile([P, S], F32, tag="ev")
                    for jc in range(nch):
                        wj = min(SC, W - jc * SC)
                        ps = ps_sc.tile([P, SC], F32, tag="pssc")
                        nc.tensor.matmul(
                            ps[:, :wj],
                            q_ro[:, h, i * P : (i + 1) * P],
                            k_ro[:, jc * SC : jc * SC + wj],
                            start=True, stop=True,
                        )
                        dst = ev[:, jc * SC : jc * SC + wj]
                        if jc == nch - 1:
                            nc.vector.tensor_add(dst, ps[:, :wj],
                                                 tri_sb[:, SC - wj : SC])
                        else:
                            if ev_dve:
                                nc.vector.tensor_copy(dst, ps[:, :wj])
                            else:
                                nc.scalar.copy(dst, ps[:, :wj])
                            ev_dve = not ev_dve
                    nc.scalar.dma_start(out[h, i * P : (i + 1) * P, 0:W], ev[:, :W])


def _host_prep(inputs_embeds, attention_mask, g, Wq, Wk):
    """Shared (core-independent) host-side constant prep."""
    x = np.asarray(inputs_embeds, dtype=np.float32).reshape(S, D)
    xb = x.astype(ml_dtypes.bfloat16)

    g32 = np.asarray(g, dtype=np.float32)
    scale = np.float32(1.0 / math.sqrt(HD))
    wq_full = (np.asarray(Wq, np.float32) * g32[:, None] * scale).astype(
        ml_dtypes.bfloat16
    )
    wk_full = (np.asarray(Wk, np.float32) * g32[:, None]).astype(ml_dtypes.bfloat16)

    pos = np.arange(S, dtype=np.float32)
    inv_freq = (1.0 / ROPE_THETA ** (np.arange(0, HD, 2, dtype=np.float32) / HD))
    freq_d = np.concatenate([inv_freq, inv_freq])  # [128], emb freq per dim d
    ang = freq_d[:, None] * pos[None, :]  # [128, S]
    cos_t = np.cos(ang).astype(ml_dtypes.bfloat16)
    sin_t = np.sin(ang)
    sin_t[:64] *= -1.0  # rotate-half sign folded into the table
    sinn_t = sin_t.astype(ml_dtypes.bfloat16)

    tri = np.zeros((P, SC), dtype=np.float32)
    blk = np.where(np.arange(P)[None, :] > np.arange(P)[:, None], MIN_F, 0.0)
    tri[:, SC - P :] = blk.astype(np.float32)

    identb = np.eye(P, dtype=ml_dtypes.bfloat16)
    identf = np.eye(P, dtype=np.float32)
    return xb, wq_full, wk_full, cos_t, sinn_t, tri, identb, identf


def _reference_numpy(inputs_embeds, attention_mask, g, Wq, Wk):
    """Fallback exact-ish path (only used if attention_mask isn't all ones)."""
    x = np.asarray(inputs_embeds, np.float32)
    var = np.mean(np.square(x), axis=-1, keepdims=True)
    h = x / np.sqrt(var + RMS_EPS) * np.asarray(g, np.float32)
    q = (h.reshape(S, D) @ np.asarray(Wq, np.float32)).reshape(B, S, H, HD)
    k = (h.reshape(S, D) @ np.asarray(Wk, np.float32)).reshape(B, S, KVH, HD)
    q = q.transpose(0, 2, 1, 3)
    k = k.transpose(0, 2, 1, 3)
    pos = np.arange(S, dtype=np.float32)
    inv_freq = 1.0 / ROPE_THETA ** (np.arange(0, HD, 2, dtype=np.float32) / HD)
    emb = np.concatenate([pos[:, None] * inv_freq[None, :]] * 2, axis=-1)
    cos, sin = np.cos(emb), np.sin(emb)

    def rope(v):
        rot = np.concatenate([-v[..., HD // 2 :], v[..., : HD // 2]], axis=-1)
        return v * cos + rot * sin

    q, k = rope(q), rope(k)
    k = np.repeat(k, H // KVH, axis=1)
    scores = np.einsum("bhqd,bhkd->bhqk", q, k) / np.float32(math.sqrt(HD))
    i = np.arange(S)[:, None]
    j = np.arange(S)[None, :]
    causal = np.where(j > i, MIN_F, 0.0).astype(np.float32)
    am = np.asarray(attention_mask, np.float32)
    pad = (causal[None, None] == 0.0) & (am[:, None, None, :] == 0.0)
    mask = np.where(pad, MIN_F, causal[None, None]).astype(np.float32)
    return (scores + mask).astype(np.float32)


last_results = None  # test.py reads exec_time_ns off this


def kernel(inputs_embeds, attention_mask, g, Wq, Wk):
    am = np.asarray(attention_mask, np.float32)
    if not np.all(am == 1.0):
        return _reference_numpy(inputs_embeds, attention_mask, g, Wq, Wk)

    xb, wq_full, wk_full, cos_t, sinn_t, tri, identb, identf = _host_prep(
        inputs_embeds, attention_mask, g, Wq, Wk
    )

    if "nc" not in _cache:
        _cache["nc"] = _build_nc()
    nc = _cache["nc"]

    in_maps = []
    for i in range(NCORES):
        in_maps.append(
            {
                "xb": xb,
                "wq": np.ascontiguousarray(
                    wq_full[:, i * HPC * HD : (i + 1) * HPC * HD]
                ),
                "wk": np.ascontiguousarray(wk_full[:, i * HD : (i + 1) * HD]),
                "cos": cos_t,
                "sinn": sinn_t,
                "tri": tri,
                "identb": identb,
                "identf": identf,
            }
        )

    global last_results
    res = run_bass_kernel_spmd(nc, in_maps, core_ids=list(range(NCORES)))
    last_results = res

    out = np.empty((B, H, S, S), dtype=np.float32)
    for i in range(NCORES):
        out[0, i * HPC : (i + 1) * HPC] = res.results[i]["out"]
    return out


# revision 17
# speedup vs baseline: 1.0738x; 1.0738x over previous
"""Trainium2 Bass kernel for nn_CustomLLamaModel (RMSNorm + QK proj + RoPE + causal QK^T).

Sharding: 8 cores, tensor-parallel over attention heads. Core i computes q heads
4i..4i+3 and kv head i (GQA groups align exactly with the 8 cores, so no
collectives are needed). Each core receives the full (bf16-cast) activations and
its weight shard, and writes its 4 heads' [2048, 2048] score matrices.

Device pipeline per core (all matmuls bf16, PSUM f32):
  - x row-tiles [128, 4096]: bn_stats/bn_aggr -> mean(x^2) -> r = rsqrt(mean+eps)
  - transpose x via PE matmuls (lhsT=x chunk, rhs=I) -> xT [4096, 512-chunk]
  - r is folded into the RoPE cos/sin tables (rope is linear, rope(r*v)=r*rope(v)),
    so projections run on the UN-normalized xT and normalization comes out in rope
  - projections: qT/kT = W^T @ xT accumulated over 32 K-chunks
  - rope: rotate-half via two SBUF->SBUF partition-shift DMAs; sign folded in table
  - scores: only lower-triangle 512-blocks are computed; the diagonal block gets a
    precomputed triangular min_f mask added at PSUM eviction; the upper-triangle
    region is written from a constant min_f SBUF tile (exact: score+min_f == min_f
    in f32); 1/sqrt(HD) and the RMSNorm gain g are folded into Wq/Wk on the host.
"""

import os
import sys

sys.path.insert(0, "/opt/trn_rl_repo")

import math
import numpy as np
import ml_dtypes

_THIS_DIR = os.path.dirname(os.path.abspath(__file__))
if _THIS_DIR not in sys.path:
    sys.path.insert(0, _THIS_DIR)

try:
    import axon_profile_shim

    axon_profile_shim.install()
except Exception:
    pass

import concourse.bass as bass
import concourse.mybir as mybir
import concourse.tile as tile
from concourse import bacc
from concourse.bass_utils import run_bass_kernel_spmd

B, S, D = 1, 2048, 4096
H, KVH, HD = 32, 8, 128
ROPE_THETA = 10000.0
RMS_EPS = 1e-5
NCORES = 8
HPC = H // NCORES  # q heads per core = 4
P = 128
NRT = S // P  # 16 row tiles
SC = 512  # seq chunk
NSC = S // SC  # 4 chunks
KO = D // P  # 32 contraction chunks
MIN_F = float(np.finfo(np.float32).min)

BF16 = mybir.dt.bfloat16
F32 = mybir.dt.float32

_cache = {}


def _build_nc():
    """Build + compile the per-core NEFF (same program for all 8 cores)."""
    nc = bacc.Bacc(
        "TRN2",
        target_bir_lowering=False,
        debug=False,
        enable_asserts=True,
        num_devices=NCORES,
    )
    xb = nc.dram_tensor("xb", [S, D], BF16, kind="ExternalInput")
    wq = nc.dram_tensor("wq", [D, HPC * HD], BF16, kind="ExternalInput")
    wk = nc.dram_tensor("wk", [D, HD], BF16, kind="ExternalInput")
    cos_d = nc.dram_tensor("cos", [P, S], BF16, kind="ExternalInput")
    sinn_d = nc.dram_tensor("sinn", [P, S], BF16, kind="ExternalInput")
    tri_d = nc.dram_tensor("tri", [P, SC], F32, kind="ExternalInput")
    identb_d = nc.dram_tensor("identb", [P, P], BF16, kind="ExternalInput")
    identf_d = nc.dram_tensor("identf", [P, P], F32, kind="ExternalInput")
    out = nc.dram_tensor("out", [HPC, S, S], F32, kind="ExternalOutput")

    with tile.TileContext(nc) as tc:
        _emit(nc, tc, xb, wq, wk, cos_d, sinn_d, tri_d, identb_d, identf_d, out)
    nc.compile()
    return nc


def _emit(nc, tc, xb, wq, wk, cos_d, sinn_d, tri_d, identb_d, identf_d, out):
    from contextlib import ExitStack

    ctx = ExitStack()
    with ctx:
        singles = ctx.enter_context(tc.tile_pool(name="singles", bufs=1))
        xrow_p = ctx.enter_context(tc.tile_pool(name="xrow", bufs=2))
        xt_p = ctx.enter_context(tc.tile_pool(name="xt", bufs=2))
        stat_p = ctx.enter_context(tc.tile_pool(name="stat", bufs=4))
        qt_p = ctx.enter_context(tc.tile_pool(name="qt", bufs=2))
        rot_p = ctx.enter_context(tc.tile_pool(name="rot", bufs=2))
        rbc_p = ctx.enter_context(tc.tile_pool(name="rbc", bufs=2))
        ev_p = ctx.enter_context(tc.tile_pool(name="ev", bufs=2))
        ps_tr = ctx.enter_context(tc.tile_pool(name="ps_tr", bufs=2, space="PSUM"))
        ps_pr = ctx.enter_context(tc.tile_pool(name="ps_pr", bufs=2, space="PSUM"))
        ps_sc = ctx.enter_context(tc.tile_pool(name="ps_sc", bufs=3, space="PSUM"))

        # ---- small constants first (x-row loads must not queue behind bulk) ----
        identb = singles.tile([P, P], BF16)
        nc.sync.dma_start(identb[:], identb_d[:])
        identf = singles.tile([P, P], F32)
        nc.sync.dma_start(identf[:], identf_d[:])
        tri_sb = singles.tile([P, SC], F32)
        nc.sync.dma_start(tri_sb[:], tri_d[:])
        minf_sb = singles.tile([P, S - P], F32)
        nc.vector.memset(minf_sb[:], MIN_F)
        eps_sb = singles.tile([P, 1], F32)
        nc.vector.memset(eps_sb[:], RMS_EPS)
        # bulk loads are emitted after chunk 0's x-row loads (same FIFO ring)
        wq_sb = singles.tile([P, KO, HPC * HD], BF16)
        wk_sb = singles.tile([P, KO, HD], BF16)
        cos_sb = singles.tile([P, S], BF16)
        sinn_sb = singles.tile([P, S], BF16)
        sq_dummy = singles.tile([P, D], BF16)  # discarded square output of stats

        r_all = singles.tile([P, NRT], F32)  # r_all[p, t] = rsqrt for seq 128t+p
        ss_all = singles.tile([P, NRT], F32)  # sum(x^2) per row
        cos_r = singles.tile([P, S], BF16)  # r-folded rope tables
        sin_r = singles.tile([P, S], BF16)
        q_ro = singles.tile([P, HPC, S], BF16)  # roped qT per head
        k_ro = singles.tile([P, S], BF16)  # roped kT
        r_row = singles.tile([1, SC], F32)
        r_psT = singles.tile([32, P], F32)

        # Constant min_f (upper-triangle) writes have no compute dependency:
        # issue them on the GPSIMD/SWDGE ring (separate FIFO from the SP ring
        # carrying input loads and the ACT ring carrying computed outputs) so
        # they fill DMA idle time instead of sitting on the critical-path tail.
        # Coalesced across the 4 heads per row-tile: dest reordered (r, h, c),
        # source re-reads the minf tile 4x via a 0-stride dim.
        def emit_const_dma(i):
            W = (i + 1) * P
            if W >= S:
                return
            for h in range(HPC):
                nc.gpsimd.dma_start(out[h, i * P : (i + 1) * P, W:S],
                                    minf_sb[:, : S - W])

        # constant-region writes, scheduled into each chunk's proj phase where
        # the DMA engines would otherwise idle (~7-8.5MB per chunk)
        CONST_SCHED = {0: [3, 4, 5], 1: [0, 6, 7], 2: [1, 8, 9, 12],
                       3: [2, 10, 11, 13, 14]}

        ev_dve = True  # alternate engines for copies

        for c in range(NSC):
            sl = slice(c * SC, (c + 1) * SC)
            xt_c = xt_p.tile([P, KO, SC], BF16, tag="xt")
            # ---- load 4 row-tiles, stats, transpose ----
            for tt in range(4):
                t = 4 * c + tt
                xrow = xrow_p.tile([P, D], BF16, tag="xrow")
                nc.sync.dma_start(xrow[:], xb[t * P : (t + 1) * P, :])
                # stats: ACT square with fused row-accumulate -> sum(x^2)
                nc.scalar.activation(
                    out=sq_dummy[:], in_=xrow[:],
                    func=mybir.ActivationFunctionType.Square,
                    accum_out=ss_all[:, t : t + 1],
                )
                # transpose this row-tile: 32 chunk transposes in groups of 4
                for dg in range(8):
                    ps = ps_tr.tile([P, 4 * P], F32, tag="pstr")
                    for u in range(4):
                        d = 4 * dg + u
                        nc.tensor.matmul(
                            ps[:, u * P : (u + 1) * P],
                            xrow[:, d * P : (d + 1) * P],
                            identb[:],
                            start=True, stop=True,
                        )
                    dst = xt_c[:, 4 * dg : 4 * dg + 4, tt * P : (tt + 1) * P]
                    src = ps[:].rearrange("p (a b) -> p a b", a=4)
                    if ev_dve:
                        nc.vector.tensor_copy(dst, src)
                    else:
                        nc.scalar.copy(dst, src)
                    ev_dve = not ev_dve

            if c == 0:
                # bulk resident loads, behind chunk 0's x rows on the SP ring
                nc.sync.dma_start(cos_sb[:], cos_d[:])
                nc.sync.dma_start(sinn_sb[:], sinn_d[:])
                nc.sync.dma_start(wq_sb[:], wq.rearrange("(ko p) m -> p ko m", p=P))
                nc.sync.dma_start(wk_sb[:], wk.rearrange("(ko p) m -> p ko m", p=P))

            # finalize stats for this chunk's 4 row-tiles in one batch
            csl = slice(4 * c, 4 * c + 4)
            std4 = stat_p.tile([P, 4], F32, tag="std4")
            nc.scalar.activation(
                out=std4[:], in_=ss_all[:, csl],
                func=mybir.ActivationFunctionType.Sqrt,
                bias=eps_sb[:], scale=1.0 / D,
            )
            nc.vector.reciprocal(out=r_all[:, csl], in_=std4[:])

            # ---- r broadcast chain for this chunk ----
            ps4full = ps_pr.tile([P, SC], F32, tag="pspr")
            ps4 = ps4full[0:4, 0:P]
            nc.tensor.matmul(ps4, r_all[:, 4 * c : 4 * c + 4], identf[:],
                             start=True, stop=True)
            nc.vector.tensor_copy(r_psT[0:4, :], ps4)
            for tt in range(4):
                nc.sync.dma_start(r_row[0:1, tt * P : (tt + 1) * P],
                                  r_psT[tt : tt + 1, :])
            r_bc = rbc_p.tile([P, SC], F32, tag="rbc")
            nc.gpsimd.partition_broadcast(r_bc[:], r_row[0:1, :])
            nc.vector.tensor_mul(cos_r[:, sl], cos_sb[:, sl], r_bc[:])
            nc.vector.tensor_mul(sin_r[:, sl], sinn_sb[:, sl], r_bc[:])
            for i in CONST_SCHED[c]:
                emit_const_dma(i)

            # ---- projections + rope ----
            # out-tiles: (wq_sb, m=0..3 -> q_ro[:, m, :]), (wk_sb -> k_ro)
            proj_list = [(wq_sb, m, q_ro[:, m, :]) for m in range(HPC)]
            proj_list.append((wk_sb, 0, k_ro[:]))
            for w_sb, m, dest in proj_list:
                ps = ps_pr.tile([P, SC], F32, tag="pspr")
                for ko in range(KO):
                    nc.tensor.matmul(
                        ps[:],
                        w_sb[:, ko, m * P : (m + 1) * P],
                        xt_c[:, ko, :],
                        start=(ko == 0), stop=(ko == KO - 1),
                    )
                qt = qt_p.tile([P, SC], BF16, tag="qt")
                nc.scalar.copy(qt[:], ps[:])  # evict f32->bf16 on ACT
                rot = rot_p.tile([P, SC], BF16, tag="rot")
                nc.sync.dma_start(rot[0:64, :], qt[64:128, :])
                nc.sync.dma_start(rot[64:128, :], qt[0:64, :])
                nc.vector.tensor_mul(rot[:], rot[:], sin_r[:, sl])
                nc.vector.tensor_mul(dest[:, sl], qt[:], cos_r[:, sl])
                nc.vector.tensor_add(dest[:, sl], dest[:, sl], rot[:])

            # ---- scores for q row-tiles of this chunk ----
            for h in range(HPC):
                for tt in range(4):
                    i = 4 * c + tt
                    W = (i + 1) * P
                    nch = (W + SC - 1) // SC
                    ev = ev_p.tile([P, S], F32, tag="ev")
                    for jc in range(nch):
                        wj = min(SC, W - jc * SC)
                        ps = ps_sc.tile([P, SC], F32, tag="pssc")
                        nc.tensor.matmul(
                            ps[:, :wj],
                            q_ro[:, h, i * P : (i + 1) * P],
                            k_ro[:, jc * SC : jc * SC + wj],
                            start=True, stop=True,
                        )
                        dst = ev[:, jc * SC : jc * SC + wj]
                        if jc == nch - 1:
                            nc.vector.tensor_add(dst, ps[:, :wj],
                                                 tri_sb[:, SC - wj : SC])
                        else:
                            if ev_dve:
                                nc.vector.tensor_copy(dst, ps[:, :wj])
                            else:
                                nc.scalar.copy(dst, ps[:, :wj])
                            ev_dve = not ev_dve
                    nc.scalar.dma_start(out[h, i * P : (i + 1) * P, 0:W], ev[:, :W])


def _host_prep(inputs_embeds, attention_mask, g, Wq, Wk):
    """Shared (core-independent) host-side constant prep."""
    x = np.asarray(inputs_embeds, dtype=np.float32).reshape(S, D)
    xb = x.astype(ml_dtypes.bfloat16)

    g32 = np.asarray(g, dtype=np.float32)
    scale = np.float32(1.0 / math.sqrt(HD))
    wq_full = (np.asarray(Wq, np.float32) * g32[:, None] * scale).astype(
        ml_dtypes.bfloat16
    )
    wk_full = (np.asarray(Wk, np.float32) * g32[:, None]).astype(ml_dtypes.bfloat16)

    pos = np.arange(S, dtype=np.float32)
    inv_freq = (1.0 / ROPE_THETA ** (np.arange(0, HD, 2, dtype=np.float32) / HD))
    freq_d = np.concatenate([inv_freq, inv_freq])  # [128], emb freq per dim d
    ang = freq_d[:, None] * pos[None, :]  # [128, S]
    cos_t = np.cos(ang).astype(ml_dtypes.bfloat16)
    sin_t = np.sin(ang)
    sin_t[:64] *= -1.0  # rotate-half sign folded into the table
    sinn_t = sin_t.astype(ml_dtypes.bfloat16)

    tri = np.zeros((P, SC), dtype=np.float32)
    blk = np.where(np.arange(P)[None, :] > np.arange(P)[:, None], MIN_F, 0.0)
    tri[:, SC - P :] = blk.astype(np.float32)

    identb = np.eye(P, dtype=ml_dtypes.bfloat16)
    identf = np.eye(P, dtype=np.float32)
    return xb, wq_full, wk_full, cos_t, sinn_t, tri, identb, identf


def _reference_numpy(inputs_embeds, attention_mask, g, Wq, Wk):
    """Fallback exact-ish path (only used if attention_mask isn't all ones)."""
    x = np.asarray(inputs_embeds, np.float32)
    var = np.mean(np.square(x), axis=-1, keepdims=True)
    h = x / np.sqrt(var + RMS_EPS) * np.asarray(g, np.float32)
    q = (h.reshape(S, D) @ np.asarray(Wq, np.float32)).reshape(B, S, H, HD)
    k = (h.reshape(S, D) @ np.asarray(Wk, np.float32)).reshape(B, S, KVH, HD)
    q = q.transpose(0, 2, 1, 3)
    k = k.transpose(0, 2, 1, 3)
    pos = np.arange(S, dtype=np.float32)
    inv_freq = 1.0 / ROPE_THETA ** (np.arange(0, HD, 2, dtype=np.float32) / HD)
    emb = np.concatenate([pos[:, None] * inv_freq[None, :]] * 2, axis=-1)
    cos, sin = np.cos(emb), np.sin(emb)

    def rope(v):
        rot = np.concatenate([-v[..., HD // 2 :], v[..., : HD // 2]], axis=-1)
        return v * cos + rot * sin

    q, k = rope(q), rope(k)
    k = np.repeat(k, H // KVH, axis=1)
    scores = np.einsum("bhqd,bhkd->bhqk", q, k) / np.float32(math.sqrt(HD))
    i = np.arange(S)[:, None]
    j = np.arange(S)[None, :]
    causal = np.where(j > i, MIN_F, 0.0).astype(np.float32)
    am = np.asarray(attention_mask, np.float32)
    pad = (causal[None, None] == 0.0) & (am[:, None, None, :] == 0.0)
    mask = np.where(pad, MIN_F, causal[None, None]).astype(np.float32)
    return (scores + mask).astype(np.float32)


last_results = None  # test.py reads exec_time_ns off this


def kernel(inputs_embeds, attention_mask, g, Wq, Wk):
    am = np.asarray(attention_mask, np.float32)
    if not np.all(am == 1.0):
        return _reference_numpy(inputs_embeds, attention_mask, g, Wq, Wk)

    xb, wq_full, wk_full, cos_t, sinn_t, tri, identb, identf = _host_prep(
        inputs_embeds, attention_mask, g, Wq, Wk
    )

    if "nc" not in _cache:
        _cache["nc"] = _build_nc()
    nc = _cache["nc"]

    in_maps = []
    for i in range(NCORES):
        in_maps.append(
            {
                "xb": xb,
                "wq": np.ascontiguousarray(
                    wq_full[:, i * HPC * HD : (i + 1) * HPC * HD]
                ),
                "wk": np.ascontiguousarray(wk_full[:, i * HD : (i + 1) * HD]),
                "cos": cos_t,
                "sinn": sinn_t,
                "tri": tri,
                "identb": identb,
                "identf": identf,
            }
        )

    global last_results
    res = run_bass_kernel_spmd(nc, in_maps, core_ids=list(range(NCORES)))
    last_results = res

    out = np.empty((B, H, S, S), dtype=np.float32)
    for i in range(NCORES):
        out[0, i * HPC : (i + 1) * HPC] = res.results[i]["out"]
    return out
